# revision 27
# baseline (speedup 1.0000x reference)
"""BiAttention (BiDAF-style) layer for Trainium2, data-parallel over batch.

Shapes (hardcoded, from the problem spec):
  encoded_passage  [B=8, P=2048, D=768] f32
  encoded_question [B=8, Q=256,  D=768] f32
  passage_mask     [B=8, P=2048] f32 (binary)
  question_mask    [B=8, Q=256]  f32 (binary)
  output           [B=8, P=2048, 4*D=3072] f32

Each of the 8 NeuronCores processes one batch element; no communication.

The kernel is DMA-bandwidth bound, so both the inputs and the output cross
HBM as fp16 (16.2 MB per core instead of 32.3 MB): passage/question are cast
to fp16 on the host, the output tensor is fp16 on-device and upcast to f32
on the host. fp16 has a 10-bit mantissa (same as the tf32-style f32r path),
keeping the end-to-end relative error ~1e-3, far under the 2e-2 gate. All
matmuls and transposes run at full PE rate in fp16, and the 16-bit dtype
doubles DVE elementwise throughput.

Masking uses the reference's own semantics: the question mask is folded into
the transposed question (masked columns of sim become exactly 0), so the
row max matches the reference's max(mask*sim), and masked entries contribute
exp(-max) ~ e^-80 ~ 0 to the softmax sum — no NEG_VAL pass over sim needed.
"""

import numpy as np

B, P, Q, D = 8, 2048, 256, 768
N_CORES = 8
EPS = 1e-07
NEG_VAL = -10000000.0  # phase-2 masking constant (f32 path)

NT = P // 128  # 16 passage tiles
DC = D // 128  # 6 contraction chunks
QC = Q // 128  # 2 question chunks


def build_nc(repeat=1):
    """Build (trace + schedule + bacc-compile) the single-core Bass program.

    repeat>1 emits the whole body N times (same buffers) — used only for
    low-noise hardware timing, never for grading.
    """
    import concourse.bass as bass
    import concourse.mybir as mybir
    import concourse.tile as tile
    import concourse.bass_isa as bass_isa
    from concourse import bacc
    from concourse.bass import ts
    from concourse.masks import make_identity

    f32 = mybir.dt.float32
    f16 = mybir.dt.float16
    Alu = mybir.AluOpType
    Act = mybir.ActivationFunctionType
    Axis = mybir.AxisListType

    nc = bacc.Bacc(
        "TRN2",
        target_bir_lowering=False,
        debug=False,
        enable_asserts=False,
        num_devices=N_CORES,
    )

    ep = nc.dram_tensor("encoded_passage", [P, D], f16, kind="ExternalInput").ap()
    eq = nc.dram_tensor("encoded_question", [Q, D], f16, kind="ExternalInput").ap()
    pmsk = nc.dram_tensor("passage_mask", [P], f32, kind="ExternalInput").ap()
    qmsk = nc.dram_tensor("question_mask", [Q], f32, kind="ExternalInput").ap()
    # device output holds chunks 1-3 only (cols 768:3072 of the final
    # output); chunk 0 is the passage itself and is placed host-side
    out = nc.dram_tensor("out", [P, 3 * D], f16, kind="ExternalOutput").ap()

    with tile.TileContext(nc) as tc:
        with (
            tc.tile_pool(name="const", bufs=1) as const,
            tc.tile_pool(name="work", bufs=5) as work,
            tc.tile_pool(name="sm", bufs=6) as sm,
            tc.tile_pool(name="small", bufs=4) as small,
            tc.tile_pool(name="store", bufs=2) as store,
            tc.tile_pool(name="psTR", bufs=3, space="PSUM") as psTR,
            tc.tile_pool(name="psSIM", bufs=2, space="PSUM") as psSIM,
            tc.tile_pool(name="psPQ", bufs=2, space="PSUM") as psPQ,
        ):
            # ---- constants / persistent tiles ----
            id_h = const.tile([128, 128], f16)
            make_identity(nc, id_h)
            id_f = const.tile([16, 16], f32)
            make_identity(nc, id_f)

            pas_all = const.tile([128, NT, D], f16)  # passage, natural layout
            qnat = const.tile([128, QC, D], f16)  # question, natural layout
            qT = const.tile([128, DC, Q], f16)  # qmask * question^T  [d, q]
            qmask_b = const.tile([128, Q], f32)  # question mask bcast over rows
            qp_all = const.tile([128, NT], f32)  # qp_similarity columns
            r_all = const.tile([128, NT], f32)  # 1/(softmax sum + eps) per tile
            qp_bc = const.tile([128, D], f16)  # qp_vector broadcast
            aT_all = const.tile([128, NT, QC, 128], f16)  # t'^T per tile
            negm1_all = const.tile([128, NT], f32)  # -max(mask*sim) per tile
            ssum_all = const.tile([128, NT], f32)  # exp-sum per tile

            # load order tuned for the startup critical path: each DMA's
            # consumer starts ~1.7us after transfer end (completion latency),
            # so the small gating loads go first and the first passage tile
            # is split out so its transposes start as early as possible
            nc.sync.dma_start(
                out=qnat[:, :, :], in_=eq.rearrange("(a q) d -> q a d", q=128)
            )
            nc.sync.dma_start(out=pas_all[:, 0:1, :], in_=ep[0:128, :])
            nc.sync.dma_start(out=qmask_b[:, :], in_=qmsk.partition_broadcast(128))
            nc.sync.dma_start(
                out=pas_all[:, 1:4, :],
                in_=ep[128:512, :].rearrange("(tt p) c -> p tt c", p=128),
            )
            pm_nat = const.tile([16, 128], f32)
            nc.sync.dma_start(
                out=pm_nat[:, :], in_=pmsk.rearrange("(t p) -> t p", p=128)
            )

            # masked question transpose: qT[:, dc, :] = qmask * qnat[:, :, dc].T
            # (the mask multiply rides along on the PSUM eviction, batched
            # over 4/2 contraction chunks per DVE op)
            for dc0, ndc in ((0, 4), (4, 2)):
                ps_q = psTR.tile([128, 8, 128], f16, tag="tr8")
                for i in range(ndc):
                    for qc in range(QC):
                        nc.tensor.transpose(
                            ps_q[:, 2 * i + qc, :],
                            qnat[:, qc, ts(dc0 + i, 128)],
                            id_h[:, :],
                        )
                qm_rep = bass.AP(
                    tensor=qmask_b.tensor,
                    offset=qmask_b.offset,
                    ap=[[Q, 128], [0, ndc], [1, Q]],
                )
                nc.vector.tensor_mul(
                    qT[:, dc0 : dc0 + ndc, :],
                    ps_q.rearrange("p (a b) c -> p a (b c)", b=QC)[:, 0:ndc, :],
                    qm_rep,
                )

            # passage mask, transposed to [p_in_tile, tile] layout
            ps_pm = psSIM.tile([128, Q], f32, tag="sim")
            nc.tensor.transpose(ps_pm[:, 0:16], pm_nat[:, :], id_f[:, :])
            pm_t = const.tile([128, NT], f32)
            nc.vector.tensor_copy(pm_t[:, :], ps_pm[:, 0:16])
            neg2 = const.tile([128, NT], f32)
            nc.vector.tensor_scalar(
                out=neg2[:, :],
                in0=pm_t[:, :],
                scalar1=-NEG_VAL,
                scalar2=NEG_VAL,
                op0=Alu.mult,
                op1=Alu.add,
            )

            # ---- phase 1: per passage-tile attention ----
            for _rep in range(repeat):
              for t in range(NT):
                  # load passage four tiles at a time (bigger DMAs run nearer
                  # to line rate); kept resident for phases 2/3
                  if t % 4 == 0 and t > 0:
                      src_quad = ep[t * 128 : (t + 4) * 128, :].rearrange(
                          "(tt p) c -> p tt c", p=128
                      )
                      nc.sync.dma_start(out=pas_all[:, t : t + 4, :], in_=src_quad)

                  # transpose passage tile: pT[:, dc, :] = pas[:, dc-chunk].T
                  pT = work.tile([128, DC, 128], f16, tag="pT")
                  ps8 = psTR.tile([128, 8, 128], f16, tag="tr8")
                  for dc in range(DC):
                      nc.tensor.transpose(
                          ps8[:, dc, :],
                          pas_all[:, t, ts(dc, 128)],
                          id_h[:, :],
                      )
                  nc.vector.tensor_copy(pT[:, :, :], ps8[:, 0:DC, :])

                  # sim tile [128, Q] in PSUM: qmask * (passage @ question^T)
                  ps_sim = psSIM.tile([128, Q], f32, tag="sim")
                  for dc in range(DC):
                      nc.tensor.matmul(
                          ps_sim[:, :],
                          lhsT=pT[:, dc, :],
                          rhs=qT[:, dc, :],
                          start=(dc == 0),
                          stop=(dc == DC - 1),
                      )

                  # max(mask*sim) is both the softmax shift and qp_similarity
                  nc.vector.tensor_reduce(
                      out=negm1_all[:, t : t + 1],
                      in_=ps_sim[:, :],
                      axis=Axis.X,
                      op=Alu.max,
                      negate=True,
                  )
                  # t' = exp(mask*sim - m1); masked entries give exp(-m1) ~ 0
                  tprime = sm.tile([128, Q], f16, tag="tp")
                  nc.scalar.activation(
                      out=tprime[:, :],
                      in_=ps_sim[:, :],
                      func=Act.Exp,
                      bias=negm1_all[:, t : t + 1],
                      scale=1.0,
                      accum_out=ssum_all[:, t : t + 1],
                  )

                  # transpose t' -> [q, p] for the pq matmul
                  for qc in range(QC):
                      nc.tensor.transpose(
                          ps8[:, DC + qc, :],
                          tprime[:, ts(qc, 128)],
                          id_h[:, :],
                      )
                  nc.scalar.copy(aT_all[:, t, :, :], ps8[:, DC : DC + 2, :])
                  if t % 4 == 3:
                      q0 = t - 3
                      nc.vector.tensor_scalar_mul(
                          qp_all[:, q0 : t + 1], negm1_all[:, q0 : t + 1], -1.0
                      )
                      se4 = small.tile([128, 4], f32, tag="se4")
                      nc.vector.tensor_scalar_add(
                          se4[:, :], ssum_all[:, q0 : t + 1], EPS
                      )
                      nc.vector.reciprocal(r_all[:, q0 : t + 1], se4[:, :])

              # ---- phase 1b: pq matmuls, products, stores (decoupled from the
              # attention chain so each engine runs long independent streams) ----
              for t in range(NT):
                  if t % 4 == 0:
                      o23p = store.tile([128, 4, 2 * D], f16, tag="o23", bufs=4)
                  j = t % 4
                  o23 = o23p[:, j, :]
                  ps_pqa = psPQ.tile([128, 512], f32, tag="pqa")
                  ps_pqb = psPQ.tile([128, 256], f32, tag="pqb", bufs=1)
                  for qc in range(QC):
                      st = qc == 0
                      sp = qc == QC - 1
                      nc.tensor.matmul(
                          ps_pqa[:, :],
                          lhsT=aT_all[:, t, qc, :],
                          rhs=qnat[:, qc, 0:512],
                          start=st,
                          stop=sp,
                      )
                      nc.tensor.matmul(
                          ps_pqb[:, :],
                          lhsT=aT_all[:, t, qc, :],
                          rhs=qnat[:, qc, 512:D],
                          start=st,
                          stop=sp,
                      )

                  # evict + normalize pq, form passage*pq, store out cols 768:2304
                  nc.scalar.mul(o23[:, 0:512], ps_pqa[:, :], r_all[:, t : t + 1])
                  nc.vector.tensor_scalar_mul(
                      o23[:, 512:D], ps_pqb[:, :], r_all[:, t : t + 1]
                  )
                  if t % 2 == 1:
                      nc.gpsimd.tensor_mul(
                          o23p[:, j - 1 : j + 1, D : 2 * D],
                          pas_all[:, t - 1 : t + 1, :],
                          o23p[:, j - 1 : j + 1, 0:D],
                      )
                      dst23 = out[(t - 1) * 128 : (t + 1) * 128, 0 : 2 * D].rearrange(
                          "(tt p) c -> p tt c", p=128
                      )
                      nc.sync.dma_start(out=dst23, in_=o23p[:, j - 1 : j + 1, :])

              # ---- phase 2: masked softmax over all P, then qp_vector ----
              im2 = sm.tile([128, NT], f32, tag="im2")
              nc.vector.tensor_mul(im2[:, :], qp_all[:, :], pm_t[:, :])
              rowmax = small.tile([128, 1], f32, tag="p2")
              nc.vector.tensor_reduce(
                  out=rowmax[:, :], in_=im2[:, :], axis=Axis.X, op=Alu.max
              )
              gmax = small.tile([128, 1], f32, tag="p2")
              nc.gpsimd.partition_all_reduce(
                  gmax[:, :], rowmax[:, :], channels=128, reduce_op=bass_isa.ReduceOp.max
              )
              neggmax = small.tile([128, 1], f32, tag="p2")
              nc.vector.tensor_scalar_mul(neggmax[:, :], gmax[:, :], -1.0)

              # t2 = unnormalized softmax weights, fp16 for the qp matmul;
              # the 1/(sum+eps) scale is applied after the matmul instead
              # (linear), so the qp matmuls overlap the sum all-reduce.
              t2 = sm.tile([128, NT], f16, tag="t2")
              s2row = small.tile([128, 1], f32, tag="p2")
              ms2 = sm.tile([128, NT], f32, tag="ms2")
              nc.vector.tensor_add(ms2[:, :], im2[:, :], neg2[:, :])
              nc.scalar.activation(
                  out=t2[:, :],
                  in_=ms2[:, :],
                  func=Act.Exp,
                  bias=neggmax[:, :],
                  scale=1.0,
                  accum_out=s2row[:, :],
              )
              gsum = small.tile([128, 1], f32, tag="p2")
              nc.gpsimd.partition_all_reduce(
                  gsum[:, :], s2row[:, :], channels=128, reduce_op=bass_isa.ReduceOp.add
              )
              ser2 = small.tile([128, 1], f32, tag="p2")
              nc.vector.tensor_scalar_add(ser2[:, :], gsum[:, :], EPS)
              r2 = small.tile([128, 1], f32, tag="p2")
              nc.vector.reciprocal(r2[:, :], ser2[:, :])

              # qp_vector[d] = (sum_p t2[p] * passage[p, d]) * r2
              ps_qpa = psPQ.tile([1, 512], f32, tag="pqa")
              ps_qpb = psPQ.tile([1, 256], f32, tag="pqb", bufs=1)
              for t in range(NT):
                  st = t == 0
                  sp = t == NT - 1
                  nc.tensor.matmul(
                      ps_qpa[:, :],
                      lhsT=t2[:, t : t + 1],
                      rhs=pas_all[:, t, 0:512],
                      start=st,
                      stop=sp,
                  )
                  nc.tensor.matmul(
                      ps_qpb[:, :],
                      lhsT=t2[:, t : t + 1],
                      rhs=pas_all[:, t, 512:D],
                      start=st,
                      stop=sp,
                  )
              qp_sb = sm.tile([1, D], f16, tag="qp_sb")
              nc.scalar.mul(qp_sb[:, 0:512], ps_qpa[:, :], r2[0:1, :])
              nc.vector.tensor_scalar_mul(qp_sb[:, 512:D], ps_qpb[:, :], r2[0:1, :])
              nc.gpsimd.partition_broadcast(qp_bc[:, :], qp_sb[:, :], channels=128)

              # ---- phase 3: passage * qp_vector products and stores ----
              qp_b2 = bass.AP(
                  tensor=qp_bc.tensor,
                  offset=qp_bc.offset,
                  ap=[[D, 128], [0, 2], [1, D]],
              )
              for h in range(NT // 2 + 1):
                  if h == 0:
                      t, w = 0, 1
                  elif h == NT // 2:
                      t, w = NT - 1, 1
                  else:
                      t, w = 2 * h - 1, 2
                  big = store.tile([128, 2, D], f16, tag="o4", bufs=4)
                  eng = nc.gpsimd if h % 2 == 1 else nc.vector
                  eng.tensor_mul(
                      big[:, 0:w, :], pas_all[:, t : t + w, :], qp_b2[:, 0:w, :]
                  )
                  dst = out[t * 128 : (t + w) * 128, 2 * D : 3 * D].rearrange(
                      "(tt p) c -> p tt c", p=128
                  )
                  nc.sync.dma_start(out=dst, in_=big[:, 0:w, :])

    nc.compile()
    return nc


_NC_CACHE = {}


def _get_nc(repeat=1):
    if repeat not in _NC_CACHE:
        _NC_CACHE[repeat] = build_nc(repeat)
    return _NC_CACHE[repeat]


def make_in_maps(encoded_passage, encoded_question, passage_mask, question_mask):
    """Per-core input dicts with the host-side fp16 cast."""
    return [
        {
            "encoded_passage": np.ascontiguousarray(
                encoded_passage[b], dtype=np.float16
            ),
            "encoded_question": np.ascontiguousarray(
                encoded_question[b], dtype=np.float16
            ),
            "passage_mask": np.ascontiguousarray(passage_mask[b], dtype=np.float32),
            "question_mask": np.ascontiguousarray(question_mask[b], dtype=np.float32),
        }
        for b in range(B)
    ]


def kernel(
    encoded_passage: np.ndarray,
    encoded_question: np.ndarray,
    passage_mask: np.ndarray,
    question_mask: np.ndarray,
) -> np.ndarray:
    from concourse.bass_utils import run_bass_kernel_spmd

    nc = _get_nc()
    in_maps = make_in_maps(
        encoded_passage, encoded_question, passage_mask, question_mask
    )
    res = run_bass_kernel_spmd(nc, in_maps, core_ids=list(range(N_CORES)))
    full = np.empty((B, P, 4 * D), dtype=np.float32)
    full[:, :, 0:D] = encoded_passage
    for b in range(B):
        full[b, :, D:] = res.results[b]["out"].astype(np.float32)
    return full
